# revision 24
# baseline (speedup 1.0000x reference)
"""Trainium2 Bass kernel for the siamese-kNN classification head.

Reference computation (B=256, N=2000, D=512, C=100):
    scores[b,n] = sigmoid(sum_d w_d * |a[b,d] - S[n,d]| + kb)
    out[b,c]    = (scores @ L)[b,c] / count_c     (0 where count_c == 0)

Strategy
--------
Data-parallel over the batch: core i handles rows 32*i .. 32*i+32, no
collectives.  Via |x| = relu(2x) - x the score splits into a nonlinear slab
relu(A''-S'') (A''=2|w|a, S''=2|w|S, bf16) plus a separable rank-2 linear
correction preloaded into PSUM (kb - w.a_b + (w.S)_n, f32r matmul).

Layout: d on partitions (4 chunks of 128), n on the free dim.  Slabs are
produced by DVE (tensor_scalar, 4x bf16 mode, ~0.74us/slab) and ACT
(activation Relu, ~1.9us/slab); the PE reduces each slab over d with a
[128,32] sliding-window stationary that is zero except column 31-b = -/+ the
per-row sign of w, accumulating row b of psc[32, seg] in PSUM.

Main optimizations over the v1 kernel (133.3us):
 1. The d-axis is HOST-PERMUTED so every partition row of chunk pair (0,1)
    and (2,3) carries two SAME-SIGN w's (sign classes padded to even size by
    reclassifying the smallest-|w| d, error ~1e-4).  For "paired" batch rows
    the two chunk slabs are then pair-added on DVE (tensor_tensor, 2x bf16)
    and the PE ingests HALF the data with a per-pair-row sign stationary.
    This rebalances the engines: PE was the 115us bottleneck with DVE at 83
    and ACT at 34; now PE/DVE/ACT all land near ~91us.
 2. More batch rows produced on ACT (11 vs 4), all of them paired.
 3. Startup: DMA issue is split across both HWDGE queues (Sync + ACT) in
    priority order, the rank-2 correction matmuls run while s2t streams in
    (they only need the tiny clhs/crhs), and phase A only touches chunk 0,
    so the PE starts ~3us earlier.
 4. PSUM segments are [512,512,512,256,208] (128-aligned): no transpose
    window straddles segments, and the small trailing segments shorten the
    close->sigmoid->transpose->label-matmul tail, which is pipelined
    per-segment.
"""

import sys

for _p in ("/opt/trn_rl_repo", "/root/.axon_site/_ro/trn_rl_repo"):
    if _p not in sys.path:
        sys.path.append(_p)

import numpy as np

B, N, D, C = 256, 2000, 512, 100
NP = 2048                  # label rows padded to 16 full chunks
NCORES = 8
BSH = B // NCORES          # 32 batch rows per core
DCH = D // 128             # 4 d-chunks
SEGL = [512, 512, 512, 256, 208]       # PSUM free-dim segments (128-aligned)
SEGO = [0, 512, 1024, 1536, 1792]      # segment start offsets
NSEG = len(SEGL)
NLAB = NP // 128           # 16 label chunks
WIN_SEG = [0, 0, 0, 0, 1, 1, 1, 1, 2, 2, 2, 2, 3, 3, 4, 4]  # 128-col window -> seg

N_ACT = 10                 # batch rows produced on ACT (always paired)
N_PAIR_DVE = 7             # DVE-produced rows that are also pair-added
# GpSimd TENSOR ops share the SBUF port with DVE: routing pair-adds there
# slowed DVE tensor ops by ~70% (735->1274ns) and regressed 117->181us.
# Software-DGE accumulating DMAs instead only use gpsimd for descriptor
# generation (~1us) and do the read-modify-write on the idle DMA fabric.
DMA_ADD_FRAC = (34, 34)    # fraction of pair-adds routed to swdge accum-DMA
PEND_KEEP = 4              # pair MM deferral depth (covers swdge latency)
N_TAILB = 3                # trailing rows emitted seg-major to stagger closes

_CACHE = {}


def _sets():
    acts = [b for b in range(32)
            if (b * N_ACT) // BSH != ((b + 1) * N_ACT) // BSH]
    dves = [b for b in range(32) if b not in acts]
    # paired DVE rows from the middle of the DVE list
    m = len(dves) // 2
    pdve = dves[m:m + N_PAIR_DVE]
    unp = [b for b in dves if b not in pdve]
    return set(acts), acts + pdve, unp


def _split_multi_waits(nc):
    """TRN2 TPB instructions encode at most ONE semaphore wait, but Tile can
    attach several; split the extras into single-wait NOPs directly before
    the instruction on the same engine (engines execute in order)."""
    from concourse import mybir

    for fn in nc.m.functions:
        for bb in fn.blocks:
            out = []
            for inst in bb.instructions:
                si = inst.sync_info
                if si is not None and si.on_wait and len(si.on_wait) > 1:
                    waits = list(si.on_wait)
                    for j, w in enumerate(waits[:-1]):
                        out.append(mybir.InstNoOp(
                            name=f"{inst.name}-sw{j}", engine=inst.engine,
                            sync_info=mybir.SyncInfo(on_wait=[w], on_update=[]),
                            ins=[], outs=[]))
                    inst.sync_info = mybir.SyncInfo(
                        on_wait=[waits[-1]], on_update=list(si.on_update))
                out.append(inst)
            bb.instructions = out


def _interleave(xs, ys):
    """Merge two lists proportionally (xs items spread among ys, ys lead)."""
    out, i, j = [], 0, 0
    while i < len(xs) or j < len(ys):
        if j >= len(ys) or (i < len(xs) and
                            i * len(ys) < j * len(xs)):
            out.append(("x", xs[i])); i += 1
        else:
            out.append(("y", ys[j])); j += 1
    return out


def _build_nc():
    import concourse.bass as bass
    import concourse.tile as tile
    from concourse import mybir

    f32 = mybir.dt.float32
    f32r = mybir.dt.float32r
    bf16 = mybir.dt.bfloat16
    nc = bass.Bass()

    s2t_d = nc.declare_dram_parameter("s2t", [D, N], bf16, isOutput=False)
    a2t_d = nc.declare_dram_parameter("a2t", [DCH, 128, BSH], f32, isOutput=False)
    # sign tables, all packed in one tensor: [128, 12, 63]
    #   [:, 0:4]  = -sign per (row, chunk)     (DVE slabs)
    #   [:, 4:8]  = +sign per (row, chunk)     (ACT slabs)
    #   [:, 8:10] = -sign per (row, pairchunk) (DVE pair slabs)
    #   [:, 10:12]= +sign per (row, pairchunk) (ACT pair slabs)
    sg_d = nc.declare_dram_parameter("sg", [128, 12, 63], bf16, isOutput=False)
    clhs_d = nc.declare_dram_parameter("clhs", [2, BSH], f32r, isOutput=False)
    crhs_d = nc.declare_dram_parameter("crhs", [2, N], f32r, isOutput=False)
    lab_d = nc.declare_dram_parameter("labels", [128, NLAB, C], bf16, isOutput=False)
    ident_d = nc.declare_dram_parameter("ident", [32, 32], bf16, isOutput=False)
    recb_d = nc.declare_dram_parameter("recb", [BSH, C], f32, isOutput=False)
    out_d = nc.declare_dram_parameter("out", [BSH, C], f32, isOutput=True)

    acts, paired, unpaired = _sets()

    with tile.TileContext(nc) as tc:
        with (
            tc.tile_pool(name="const", bufs=1) as const,
            tc.tile_pool(name="dslab", bufs=10) as dpool,
            tc.tile_pool(name="aslab", bufs=6) as apool,
            tc.tile_pool(name="qslab", bufs=8) as qpool,
            tc.tile_pool(name="bank", bufs=8, space="PSUM") as bankp,
        ):
            # ---- constant loads: split across both HWDGE queues (Sync+ACT),
            # smallest/most-critical first.  Each dma_start costs ~650ns of
            # serial issue time on its queue.
            clhs = const.tile([2, BSH], f32r, name="clhs", tag="clhs")
            nc.sync.dma_start(clhs[:], clhs_d[:])
            crhs = const.tile([2, N], f32r, name="crhs", tag="crhs")
            nc.sync.dma_start(crhs[:], crhs_d[:])
            a2t = const.tile([128, DCH * BSH], f32, name="a2t", tag="a2t")
            nc.scalar.dma_start(
                a2t[:].rearrange("p (c b) -> p c b", c=DCH),
                a2t_d[:].rearrange("c p b -> p c b"),
            )
            sg = const.tile([128, 12, 63], bf16, name="sg", tag="sg")
            nc.scalar.dma_start(sg[:], sg_d[:])
            # chunk-0 tile filled by two parallel DMAs (one per queue)
            s2tv = [
                const.tile([128, N], bf16, name=f"s2t{c}", tag=f"s2t{c}")
                for c in range(DCH)
            ]
            nc.sync.dma_start(s2tv[0][:, 0:512], s2t_d[0:128, 0:512])
            nc.scalar.dma_start(s2tv[0][:, 512:1024], s2t_d[0:128, 512:1024])
            nc.sync.dma_start(s2tv[0][:, 1024:1536], s2t_d[0:128, 1024:1536])
            nc.scalar.dma_start(s2tv[0][:, 1536:N], s2t_d[0:128, 1536:N])
            nc.sync.dma_start(s2tv[1][:], s2t_d[128:256, :])
            nc.scalar.dma_start(s2tv[2][:], s2t_d[256:384, :])
            nc.sync.dma_start(s2tv[3][:], s2t_d[384:512, :])
            ident = const.tile([32, 32], bf16, name="ident", tag="ident")
            nc.scalar.dma_start(ident[:], ident_d[:])
            labs = const.tile([128, NLAB, C], bf16, name="labs", tag="labs")
            nc.sync.dma_start(labs[:], lab_d[:])
            recb = const.tile([BSH, C], f32, name="recb", tag="recb")
            nc.sync.dma_start(recb[:], recb_d[:])

            # ---- rank-2 corrections: first PE work, overlaps s2t transfer
            psc = [
                bankp.tile([BSH, SEGL[s]], f32, name=f"psc{s}", tag="bank")
                for s in range(NSEG)
            ]
            for s in range(NSEG):
                nc.tensor.matmul(
                    psc[s][:], clhs[:], crhs[:, SEGO[s]:SEGO[s] + SEGL[s]],
                    start=True, stop=False,
                )
            # ramp fillers: keep the PE p-state climbing while s2t chunk 0 is
            # still in flight (a >100ns idle drops the clock back to 1.2GHz
            # for the next 3us of work)
            scratch = bankp.tile([2, 512], f32, name="scratch", tag="bank")
            for _ in range(5):
                nc.tensor.matmul(
                    scratch[:], clhs[:, 0:2], crhs[:, 0:512],
                    start=True, stop=True, skip_group_check=True,
                )

            def produce(ch, b, into=None):
                if b in acts:
                    slab = into if into is not None else apool.tile(
                        [128, N], bf16, name="aslab", tag="aslab")
                    nc.scalar.activation(
                        slab[:], s2tv[ch][:],
                        mybir.ActivationFunctionType.Relu,
                        bias=a2t[:, ch * BSH + b: ch * BSH + b + 1],
                        scale=-1.0,
                    )
                else:
                    slab = into if into is not None else dpool.tile(
                        [128, N], bf16, name="dslab", tag="dslab")
                    nc.vector.tensor_scalar(
                        slab[:], s2tv[ch][:],
                        a2t[:, ch * BSH + b: ch * BSH + b + 1], 0.0,
                        mybir.AluOpType.subtract, mybir.AluOpType.min,
                    )
                return slab

            def emit_mms(slab, tcol, b, last=False):
                """5 segment matmuls reducing `slab` into psc via the sliding
                sign window in column tcol of the sign-table tile."""
                lhs = sg[:, tcol, 31 - b: 63 - b]
                for s in range(NSEG):
                    nc.tensor.matmul(
                        psc[s][:], lhs,
                        slab[:, SEGO[s]:SEGO[s] + SEGL[s]],
                        start=False, stop=last,
                        skip_group_check=True,
                    )

            pend = []
            nadd = [0]

            def flush(keep=0):
                while len(pend) > keep:
                    q, tcol, b = pend.pop(0)
                    emit_mms(q, tcol, b)

            def emit_pair(cA, cB, b):
                """Produce both chunk slabs and combine them into one q tile,
                via swdge accumulating DMA (idle fabric) or DVE tensor_add."""
                i, (num, den) = nadd[0], DMA_ADD_FRAC
                nadd[0] += 1
                if (i * num) // den != ((i + 1) * num) // den:
                    sA = produce(cA, b)
                    q = qpool.tile([128, N], bf16, name="qslab", tag="qslab")
                    produce(cB, b, into=q)
                    nc.gpsimd.dma_start(q[:], sA[:],
                                        accum_op=mybir.AluOpType.add)
                else:
                    sA = produce(cA, b)
                    sB = produce(cB, b)
                    q = qpool.tile([128, N], bf16, name="qslab", tag="qslab")
                    nc.vector.tensor_add(q[:], sA[:], sB[:])
                return q

            # ---- main loop: phase A (unpaired ch0), then two chunk-pair
            # phases with paired rows interleaved among unpaired rows
            for b in unpaired:
                emit_mms(produce(0, b), 0, b)
            tailb = set(unpaired[-N_TAILB:])
            order = _interleave(paired, unpaired[:-1]) + [("y", unpaired[-1])]
            tail_groups = []
            for ph, (cA, cB) in enumerate([(0, 1), (2, 3)]):
                for kind, b in order:
                    if kind == "x":          # paired row: both chunks + add
                        q = emit_pair(cA, cB, b)
                        pend.append((q, (10 if b in acts else 8) + ph, b))
                    elif ph == 1:            # unpaired row: chunk 2 then 3
                        if b in tailb:       # defer: emitted seg-major below
                            tail_groups.append((produce(2, b), 2, b))
                            tail_groups.append((produce(3, b), 3, b))
                            if b == unpaired[-1]:
                                flush(0)
                            continue
                        slab = produce(2, b)
                        flush(PEND_KEEP)
                        emit_mms(slab, 2, b)
                        slab = produce(3, b)
                        flush(PEND_KEEP)
                        emit_mms(slab, 3, b)
                    else:                    # unpaired row: chunk 1
                        slab = produce(1, b)
                        flush(PEND_KEEP)
                        emit_mms(slab, 1, b)
            assert not pend, "pair groups left unflushed"
            # trailing groups seg-major: psc[s] closes stagger ~2us so the
            # serial sigmoid chain starts before the PE fully drains
            for s in range(NSEG):
                for gi, (slab, tcol, b) in enumerate(tail_groups):
                    nc.tensor.matmul(
                        psc[s][:], sg[:, tcol, 31 - b: 63 - b],
                        slab[:, SEGO[s]:SEGO[s] + SEGL[s]],
                        start=False, stop=(gi == len(tail_groups) - 1),
                        skip_group_check=True,
                    )

            # ---- tail: per-seg sigmoid -> transposes -> copies -> label MMs
            ssig = const.tile([BSH, N], bf16, name="ssig", tag="ssig")
            tpall = bankp.tile([128, NLAB * BSH], bf16, name="tpall", tag="bank")
            sct = const.tile([128, NLAB * BSH], bf16, name="sct", tag="sct")
            out_ps = bankp.tile([BSH, C], f32, name="out_ps", tag="bank")
            for s in range(NSEG):
                nc.scalar.activation(
                    ssig[:, SEGO[s]:SEGO[s] + SEGL[s]], psc[s][:],
                    mybir.ActivationFunctionType.Sigmoid,
                )
                wins = [k for k in range(NLAB) if WIN_SEG[k] == s]
                for k in wins:
                    pk = min(128, N - 128 * k)
                    nc.tensor.transpose(
                        tpall[:pk, BSH * k: BSH * k + BSH],
                        ssig[:, 128 * k: 128 * k + pk], ident[:],
                    )
                lo, hi = BSH * wins[0], BSH * wins[-1] + BSH
                nc.vector.tensor_copy(sct[:, lo:hi], tpall[:, lo:hi])
            # all 16 label matmuls after the transposes: windows 0-13 stream
            # while the seg-4 copy completes, instead of five serial
            # transpose->copy->matmul round-trips on the in-order PE
            for k in range(NLAB):
                pk = min(128, N - 128 * k)
                nc.tensor.matmul(
                    out_ps[:], sct[:pk, BSH * k: BSH * k + BSH],
                    labs[:pk, k, :],
                    start=(k == 0), stop=(k == NLAB - 1),
                )

            # ---- divide by counts, write out ----
            out_s = const.tile([BSH, C], f32, name="out_s", tag="out_s")
            nc.vector.tensor_mul(out_s[:], out_ps[:], recb[:])
            nc.sync.dma_start(out_d[:], out_s[:])

    _split_multi_waits(nc)
    return nc


def _prep_host(inputs, support_tensors, support_labels, kernel_w, kernel_b):
    import ml_dtypes

    bf16 = ml_dtypes.bfloat16
    a = np.asarray(inputs, dtype=np.float32)
    S = np.asarray(support_tensors, dtype=np.float32)
    L = np.asarray(support_labels, dtype=np.float32)
    w = np.asarray(kernel_w, dtype=np.float32)
    kb = np.float32(np.asarray(kernel_b, dtype=np.float32))

    # ---- sign classes, padded to even sizes by reclassifying the smallest
    # |w| element of the odd class (relu-sign error ~|w_min|, negligible)
    sgn = np.where(w >= 0, 1.0, -1.0).astype(np.float32)
    pos = np.where(sgn > 0)[0]
    if len(pos) % 2 == 1:
        mv = pos[np.argmin(np.abs(w[pos]))]
        sgn[mv] = -1.0
    pos = np.where(sgn > 0)[0]
    neg = np.where(sgn < 0)[0]
    # pairs of same-sign d's; 256 total
    pairs = np.concatenate([pos.reshape(-1, 2), neg.reshape(-1, 2)], axis=0)
    assert pairs.shape == (256, 2)
    # permutation: chunk0 = pairs[0:128,0], chunk1 = pairs[0:128,1], etc.
    perm = np.concatenate([
        pairs[0:128, 0], pairs[0:128, 1], pairs[128:256, 0], pairs[128:256, 1]
    ])
    assert np.array_equal(np.sort(perm), np.arange(D))

    aw = 2.0 * np.abs(w)
    s2t = np.ascontiguousarray((S * aw[None, :]).T[perm]).astype(bf16)  # [D, N]
    wS = (S @ w).astype(np.float32)                                # [N]
    wa = (a @ w).astype(np.float32)                                # [B]
    a2 = (a * aw[None, :])[:, perm]                                # [B, D] permuted

    # sign tables [128, 12, 63]: col 31 = value, layout per _build_nc
    sgn_chunks = sgn[perm].reshape(DCH, 128).T                     # [128, 4]
    pair_sgn = sgn[pairs[:, 0]].reshape(2, 128).T                  # [128, 2]
    sg_all = np.zeros((128, 12, 63), dtype=np.float32)
    sg_all[:, 0:4, 31] = -sgn_chunks
    sg_all[:, 4:8, 31] = sgn_chunks
    sg_all[:, 8:10, 31] = -pair_sgn
    sg_all[:, 10:12, 31] = pair_sgn

    crhs = np.empty((2, N), dtype=np.float32)
    crhs[0] = 1.0
    crhs[1] = wS
    labp = np.zeros((NP, C), dtype=np.float32)
    labp[:N] = L
    labp = np.ascontiguousarray(
        labp.reshape(NLAB, 128, C).transpose(1, 0, 2)).astype(bf16)
    ident = np.eye(32, dtype=bf16)
    counts = L.sum(axis=0)
    recip = np.where(counts != 0, 1.0 / np.maximum(counts, 1e-30), 0.0)
    recb = np.broadcast_to(recip.astype(np.float32), (BSH, C)).copy()

    shared = {
        "s2t": s2t, "sg": sg_all.astype(bf16),
        "crhs": crhs, "labels": labp, "ident": ident, "recb": recb,
    }
    in_maps = []
    for c in range(NCORES):
        rows = slice(BSH * c, BSH * (c + 1))
        a2t_c = np.ascontiguousarray(
            a2[rows].T.reshape(DCH, 128, BSH))                     # [DCH,128,BSH]
        clhs_c = np.empty((2, BSH), dtype=np.float32)
        clhs_c[0] = kb - wa[rows]
        clhs_c[1] = 1.0
        in_maps.append(dict(shared, a2t=a2t_c, clhs=clhs_c))
    return in_maps


def kernel(**inputs) -> np.ndarray:
    from concourse.bass_utils import run_bass_kernel_spmd

    if "nc" not in _CACHE:
        _CACHE["nc"] = _build_nc()
    nc = _CACHE["nc"]

    in_maps = _prep_host(
        inputs["inputs"], inputs["support_tensors"], inputs["support_labels"],
        inputs["kernel_w"], inputs["kernel_b"],
    )
    res = run_bass_kernel_spmd(nc, in_maps, list(range(NCORES)))
    return np.concatenate([res.results[i]["out"] for i in range(NCORES)], axis=0)


# revision 26
# speedup vs baseline: 1.0224x; 1.0224x over previous
"""Trainium2 Bass kernel for the siamese-kNN classification head.

Reference computation (B=256, N=2000, D=512, C=100):
    scores[b,n] = sigmoid(sum_d w_d * |a[b,d] - S[n,d]| + kb)
    out[b,c]    = (scores @ L)[b,c] / count_c     (0 where count_c == 0)

Strategy
--------
Data-parallel over the batch: core i handles rows 32*i .. 32*i+32, no
collectives.  Via |x| = relu(2x) - x the score splits into a nonlinear slab
relu(A''-S'') (A''=2|w|a, S''=2|w|S, bf16) plus a separable rank-2 linear
correction preloaded into PSUM (kb - w.a_b + (w.S)_n, f32r matmul).

Layout: d on partitions (4 chunks of 128), n on the free dim.  Slabs are
produced by DVE (tensor_scalar, 4x bf16 mode, ~0.74us/slab) and ACT
(activation Relu, ~1.9us/slab); the PE reduces each slab over d with a
[128,32] sliding-window stationary that is zero except column 31-b = -/+ the
per-row sign of w, accumulating row b of psc[32, seg] in PSUM.

Main optimizations over the v1 kernel (133.3us):
 1. The d-axis is HOST-PERMUTED so every partition row of chunk pair (0,1)
    and (2,3) carries two SAME-SIGN w's (sign classes padded to even size by
    reclassifying the smallest-|w| d, error ~1e-4).  For "paired" batch rows
    the two chunk slabs are then pair-added on DVE (tensor_tensor, 2x bf16)
    and the PE ingests HALF the data with a per-pair-row sign stationary.
    This rebalances the engines: PE was the 115us bottleneck with DVE at 83
    and ACT at 34; now PE/DVE/ACT all land near ~91us.
 2. More batch rows produced on ACT (11 vs 4), all of them paired.
 3. Startup: DMA issue is split across both HWDGE queues (Sync + ACT) in
    priority order, the rank-2 correction matmuls run while s2t streams in
    (they only need the tiny clhs/crhs), and phase A only touches chunk 0,
    so the PE starts ~3us earlier.
 4. PSUM segments are [512,512,512,256,208] (128-aligned): no transpose
    window straddles segments, and the small trailing segments shorten the
    close->sigmoid->transpose->label-matmul tail, which is pipelined
    per-segment.
"""

import sys

for _p in ("/opt/trn_rl_repo", "/root/.axon_site/_ro/trn_rl_repo"):
    if _p not in sys.path:
        sys.path.append(_p)

import numpy as np

B, N, D, C = 256, 2000, 512, 100
NP = 2048                  # label rows padded to 16 full chunks
NCORES = 8
BSH = B // NCORES          # 32 batch rows per core
DCH = D // 128             # 4 d-chunks
SEGL = [512, 512, 512, 256, 208]       # PSUM free-dim segments (128-aligned)
SEGO = [0, 512, 1024, 1536, 1792]      # segment start offsets
NSEG = len(SEGL)
NLAB = NP // 128           # 16 label chunks
WIN_SEG = [0, 0, 0, 0, 1, 1, 1, 1, 2, 2, 2, 2, 3, 3, 4, 4]  # 128-col window -> seg

N_ACT = 11                 # batch rows produced on ACT (always paired)
N_PAIR_DVE = 3             # DVE-produced rows that are also pair-added
# GpSimd TENSOR ops share the SBUF port with DVE: routing pair-adds there
# slowed DVE tensor ops by ~70% (735->1274ns) and regressed 117->181us.
# Software-DGE accumulating DMAs only use gpsimd for descriptor generation,
# but the software queue services ~3.6us/add — 34 adds oversubscribed it
# (135us); cap at 8 so the queue drains within the kernel window.
DMA_ADD_FRAC = (8, 28)     # fraction of pair-adds routed to swdge accum-DMA
PEND_KEEP = 3              # pair MM deferral depth (covers swdge latency)
N_TAILB = 3                # trailing rows emitted seg-major to stagger closes

_CACHE = {}


def _sets():
    acts = [b for b in range(32)
            if (b * N_ACT) // BSH != ((b + 1) * N_ACT) // BSH]
    dves = [b for b in range(32) if b not in acts]
    # paired DVE rows from the middle of the DVE list
    m = len(dves) // 2
    pdve = dves[m:m + N_PAIR_DVE]
    unp = [b for b in dves if b not in pdve]
    return set(acts), acts + pdve, unp


def _split_multi_waits(nc):
    """TRN2 TPB instructions encode at most ONE semaphore wait, but Tile can
    attach several; split the extras into single-wait NOPs directly before
    the instruction on the same engine (engines execute in order)."""
    from concourse import mybir

    for fn in nc.m.functions:
        for bb in fn.blocks:
            out = []
            for inst in bb.instructions:
                si = inst.sync_info
                if si is not None and si.on_wait and len(si.on_wait) > 1:
                    waits = list(si.on_wait)
                    for j, w in enumerate(waits[:-1]):
                        out.append(mybir.InstNoOp(
                            name=f"{inst.name}-sw{j}", engine=inst.engine,
                            sync_info=mybir.SyncInfo(on_wait=[w], on_update=[]),
                            ins=[], outs=[]))
                    inst.sync_info = mybir.SyncInfo(
                        on_wait=[waits[-1]], on_update=list(si.on_update))
                out.append(inst)
            bb.instructions = out


def _interleave(xs, ys):
    """Merge two lists proportionally (xs items spread among ys, ys lead)."""
    out, i, j = [], 0, 0
    while i < len(xs) or j < len(ys):
        if j >= len(ys) or (i < len(xs) and
                            i * len(ys) < j * len(xs)):
            out.append(("x", xs[i])); i += 1
        else:
            out.append(("y", ys[j])); j += 1
    return out


def _build_nc():
    import concourse.bass as bass
    import concourse.tile as tile
    from concourse import mybir

    f32 = mybir.dt.float32
    f32r = mybir.dt.float32r
    bf16 = mybir.dt.bfloat16
    nc = bass.Bass()

    s2t_d = nc.declare_dram_parameter("s2t", [D, N], bf16, isOutput=False)
    a2t_d = nc.declare_dram_parameter("a2t", [DCH, 128, BSH], f32, isOutput=False)
    # sign tables, all packed in one tensor: [128, 12, 63]
    #   [:, 0:4]  = -sign per (row, chunk)     (DVE slabs)
    #   [:, 4:8]  = +sign per (row, chunk)     (ACT slabs)
    #   [:, 8:10] = -sign per (row, pairchunk) (DVE pair slabs)
    #   [:, 10:12]= +sign per (row, pairchunk) (ACT pair slabs)
    sg_d = nc.declare_dram_parameter("sg", [128, 12, 63], bf16, isOutput=False)
    clhs_d = nc.declare_dram_parameter("clhs", [2, BSH], f32r, isOutput=False)
    crhs_d = nc.declare_dram_parameter("crhs", [2, N], f32r, isOutput=False)
    lab_d = nc.declare_dram_parameter("labels", [128, NLAB, C], bf16, isOutput=False)
    ident_d = nc.declare_dram_parameter("ident", [32, 32], bf16, isOutput=False)
    recb_d = nc.declare_dram_parameter("recb", [BSH, C], f32, isOutput=False)
    out_d = nc.declare_dram_parameter("out", [BSH, C], f32, isOutput=True)

    acts, paired, unpaired = _sets()

    with tile.TileContext(nc) as tc:
        with (
            tc.tile_pool(name="const", bufs=1) as const,
            tc.tile_pool(name="dslab", bufs=10) as dpool,
            tc.tile_pool(name="aslab", bufs=6) as apool,
            tc.tile_pool(name="qslab", bufs=8) as qpool,
            tc.tile_pool(name="bank", bufs=8, space="PSUM") as bankp,
        ):
            # ---- constant loads: split across both HWDGE queues (Sync+ACT),
            # smallest/most-critical first.  Each dma_start costs ~650ns of
            # serial issue time on its queue.
            clhs = const.tile([2, BSH], f32r, name="clhs", tag="clhs")
            nc.sync.dma_start(clhs[:], clhs_d[:])
            crhs = const.tile([2, N], f32r, name="crhs", tag="crhs")
            nc.scalar.dma_start(crhs[:], crhs_d[:])
            # chunk-0 tile filled by four parallel DMAs (two per queue)
            s2tv = [
                const.tile([128, N], bf16, name=f"s2t{c}", tag=f"s2t{c}")
                for c in range(DCH)
            ]
            nc.sync.dma_start(s2tv[0][:, 0:512], s2t_d[0:128, 0:512])
            a2t = const.tile([128, DCH * BSH], f32, name="a2t", tag="a2t")
            nc.scalar.dma_start(
                a2t[:].rearrange("p (c b) -> p c b", c=DCH),
                a2t_d[:].rearrange("c p b -> p c b"),
            )
            nc.sync.dma_start(s2tv[0][:, 1024:1536], s2t_d[0:128, 1024:1536])
            nc.scalar.dma_start(s2tv[0][:, 512:1024], s2t_d[0:128, 512:1024])
            nc.sync.dma_start(s2tv[1][:], s2t_d[128:256, :])
            nc.scalar.dma_start(s2tv[0][:, 1536:N], s2t_d[0:128, 1536:N])
            sg = const.tile([128, 12, 63], bf16, name="sg", tag="sg")
            nc.scalar.dma_start(sg[:], sg_d[:])
            nc.sync.dma_start(s2tv[3][:], s2t_d[384:512, :])
            nc.scalar.dma_start(s2tv[2][:], s2t_d[256:384, :])
            ident = const.tile([32, 32], bf16, name="ident", tag="ident")
            nc.scalar.dma_start(ident[:], ident_d[:])
            labs = const.tile([128, NLAB, C], bf16, name="labs", tag="labs")
            nc.sync.dma_start(labs[:], lab_d[:])
            recb = const.tile([BSH, C], f32, name="recb", tag="recb")
            nc.sync.dma_start(recb[:], recb_d[:])

            # ---- rank-2 corrections: first PE work, overlaps s2t transfer
            psc = [
                bankp.tile([BSH, SEGL[s]], f32, name=f"psc{s}", tag="bank")
                for s in range(NSEG)
            ]
            for s in range(NSEG):
                nc.tensor.matmul(
                    psc[s][:], clhs[:], crhs[:, SEGO[s]:SEGO[s] + SEGL[s]],
                    start=True, stop=False,
                )
            # ramp fillers: keep the PE p-state climbing while s2t chunk 0 is
            # still in flight (a >100ns idle drops the clock back to 1.2GHz
            # for the next 3us of work)
            scratch = bankp.tile([2, 512], f32, name="scratch", tag="bank")
            for _ in range(5):
                nc.tensor.matmul(
                    scratch[:], clhs[:, 0:2], crhs[:, 0:512],
                    start=True, stop=True, skip_group_check=True,
                )

            def produce(ch, b, into=None):
                if b in acts:
                    slab = into if into is not None else apool.tile(
                        [128, N], bf16, name="aslab", tag="aslab")
                    nc.scalar.activation(
                        slab[:], s2tv[ch][:],
                        mybir.ActivationFunctionType.Relu,
                        bias=a2t[:, ch * BSH + b: ch * BSH + b + 1],
                        scale=-1.0,
                    )
                else:
                    slab = into if into is not None else dpool.tile(
                        [128, N], bf16, name="dslab", tag="dslab")
                    nc.vector.tensor_scalar(
                        slab[:], s2tv[ch][:],
                        a2t[:, ch * BSH + b: ch * BSH + b + 1], 0.0,
                        mybir.AluOpType.subtract, mybir.AluOpType.min,
                    )
                return slab

            def emit_mms(slab, tcol, b, last=False):
                """5 segment matmuls reducing `slab` into psc via the sliding
                sign window in column tcol of the sign-table tile."""
                lhs = sg[:, tcol, 31 - b: 63 - b]
                for s in range(NSEG):
                    nc.tensor.matmul(
                        psc[s][:], lhs,
                        slab[:, SEGO[s]:SEGO[s] + SEGL[s]],
                        start=False, stop=last,
                        skip_group_check=True,
                    )

            pend = []
            nadd = [0]

            def flush(keep=0):
                while len(pend) > keep:
                    q, tcol, b = pend.pop(0)
                    emit_mms(q, tcol, b)

            def emit_pair(cA, cB, b):
                """Produce both chunk slabs and combine them into one q tile,
                via swdge accumulating DMA (idle fabric) or DVE tensor_add."""
                i, (num, den) = nadd[0], DMA_ADD_FRAC
                nadd[0] += 1
                if (i * num) // den != ((i + 1) * num) // den:
                    sA = produce(cA, b)
                    q = qpool.tile([128, N], bf16, name="qslab", tag="qslab")
                    produce(cB, b, into=q)
                    nc.gpsimd.dma_start(q[:], sA[:],
                                        accum_op=mybir.AluOpType.add)
                else:
                    sA = produce(cA, b)
                    sB = produce(cB, b)
                    q = qpool.tile([128, N], bf16, name="qslab", tag="qslab")
                    nc.vector.tensor_add(q[:], sA[:], sB[:])
                return q

            # ---- main loop: phase A (unpaired ch0), then two chunk-pair
            # phases with paired rows interleaved among unpaired rows
            for b in unpaired:
                emit_mms(produce(0, b), 0, b)
            tailb = set(unpaired[-N_TAILB:])
            order = _interleave(paired, unpaired[:-1]) + [("y", unpaired[-1])]
            tail_groups = []
            for ph, (cA, cB) in enumerate([(0, 1), (2, 3)]):
                for kind, b in order:
                    if kind == "x":          # paired row: both chunks + add
                        q = emit_pair(cA, cB, b)
                        pend.append((q, (10 if b in acts else 8) + ph, b))
                    elif ph == 1:            # unpaired row: chunk 2 then 3
                        if b in tailb:       # defer: emitted seg-major below
                            tail_groups.append((produce(2, b), 2, b))
                            tail_groups.append((produce(3, b), 3, b))
                            if b == unpaired[-1]:
                                flush(0)
                            continue
                        slab = produce(2, b)
                        flush(PEND_KEEP)
                        emit_mms(slab, 2, b)
                        slab = produce(3, b)
                        flush(PEND_KEEP)
                        emit_mms(slab, 3, b)
                    else:                    # unpaired row: chunk 1
                        slab = produce(1, b)
                        flush(PEND_KEEP)
                        emit_mms(slab, 1, b)
            assert not pend, "pair groups left unflushed"
            # trailing groups seg-major: psc[s] closes stagger ~2us so the
            # serial sigmoid chain starts before the PE fully drains
            for s in range(NSEG):
                for gi, (slab, tcol, b) in enumerate(tail_groups):
                    nc.tensor.matmul(
                        psc[s][:], sg[:, tcol, 31 - b: 63 - b],
                        slab[:, SEGO[s]:SEGO[s] + SEGL[s]],
                        start=False, stop=(gi == len(tail_groups) - 1),
                        skip_group_check=True,
                    )

            # ---- tail: per-seg sigmoid -> transposes -> copies -> label MMs
            ssig = const.tile([BSH, N], bf16, name="ssig", tag="ssig")
            tpall = bankp.tile([128, NLAB * BSH], bf16, name="tpall", tag="bank")
            sct = const.tile([128, NLAB * BSH], bf16, name="sct", tag="sct")
            out_ps = bankp.tile([BSH, C], f32, name="out_ps", tag="bank")
            for s in range(NSEG):
                nc.scalar.activation(
                    ssig[:, SEGO[s]:SEGO[s] + SEGL[s]], psc[s][:],
                    mybir.ActivationFunctionType.Sigmoid,
                )
                wins = [k for k in range(NLAB) if WIN_SEG[k] == s]
                for k in wins:
                    pk = min(128, N - 128 * k)
                    nc.tensor.transpose(
                        tpall[:pk, BSH * k: BSH * k + BSH],
                        ssig[:, 128 * k: 128 * k + pk], ident[:],
                    )
                lo, hi = BSH * wins[0], BSH * wins[-1] + BSH
                nc.vector.tensor_copy(sct[:, lo:hi], tpall[:, lo:hi])
            # all 16 label matmuls after the transposes: windows 0-13 stream
            # while the seg-4 copy completes, instead of five serial
            # transpose->copy->matmul round-trips on the in-order PE
            for k in range(NLAB):
                pk = min(128, N - 128 * k)
                nc.tensor.matmul(
                    out_ps[:], sct[:pk, BSH * k: BSH * k + BSH],
                    labs[:pk, k, :],
                    start=(k == 0), stop=(k == NLAB - 1),
                )

            # ---- divide by counts, write out ----
            out_s = const.tile([BSH, C], f32, name="out_s", tag="out_s")
            nc.vector.tensor_mul(out_s[:], out_ps[:], recb[:])
            nc.sync.dma_start(out_d[:], out_s[:])

    _split_multi_waits(nc)
    return nc


def _prep_host(inputs, support_tensors, support_labels, kernel_w, kernel_b):
    import ml_dtypes

    bf16 = ml_dtypes.bfloat16
    a = np.asarray(inputs, dtype=np.float32)
    S = np.asarray(support_tensors, dtype=np.float32)
    L = np.asarray(support_labels, dtype=np.float32)
    w = np.asarray(kernel_w, dtype=np.float32)
    kb = np.float32(np.asarray(kernel_b, dtype=np.float32))

    # ---- sign classes, padded to even sizes by reclassifying the smallest
    # |w| element of the odd class (relu-sign error ~|w_min|, negligible)
    sgn = np.where(w >= 0, 1.0, -1.0).astype(np.float32)
    pos = np.where(sgn > 0)[0]
    if len(pos) % 2 == 1:
        mv = pos[np.argmin(np.abs(w[pos]))]
        sgn[mv] = -1.0
    pos = np.where(sgn > 0)[0]
    neg = np.where(sgn < 0)[0]
    # pairs of same-sign d's; 256 total
    pairs = np.concatenate([pos.reshape(-1, 2), neg.reshape(-1, 2)], axis=0)
    assert pairs.shape == (256, 2)
    # permutation: chunk0 = pairs[0:128,0], chunk1 = pairs[0:128,1], etc.
    perm = np.concatenate([
        pairs[0:128, 0], pairs[0:128, 1], pairs[128:256, 0], pairs[128:256, 1]
    ])
    assert np.array_equal(np.sort(perm), np.arange(D))

    aw = 2.0 * np.abs(w)
    s2t = np.ascontiguousarray((S * aw[None, :]).T[perm]).astype(bf16)  # [D, N]
    wS = (S @ w).astype(np.float32)                                # [N]
    wa = (a @ w).astype(np.float32)                                # [B]
    a2 = (a * aw[None, :])[:, perm]                                # [B, D] permuted

    # sign tables [128, 12, 63]: col 31 = value, layout per _build_nc
    sgn_chunks = sgn[perm].reshape(DCH, 128).T                     # [128, 4]
    pair_sgn = sgn[pairs[:, 0]].reshape(2, 128).T                  # [128, 2]
    sg_all = np.zeros((128, 12, 63), dtype=np.float32)
    sg_all[:, 0:4, 31] = -sgn_chunks
    sg_all[:, 4:8, 31] = sgn_chunks
    sg_all[:, 8:10, 31] = -pair_sgn
    sg_all[:, 10:12, 31] = pair_sgn

    crhs = np.empty((2, N), dtype=np.float32)
    crhs[0] = 1.0
    crhs[1] = wS
    labp = np.zeros((NP, C), dtype=np.float32)
    labp[:N] = L
    labp = np.ascontiguousarray(
        labp.reshape(NLAB, 128, C).transpose(1, 0, 2)).astype(bf16)
    ident = np.eye(32, dtype=bf16)
    counts = L.sum(axis=0)
    recip = np.where(counts != 0, 1.0 / np.maximum(counts, 1e-30), 0.0)
    recb = np.broadcast_to(recip.astype(np.float32), (BSH, C)).copy()

    shared = {
        "s2t": s2t, "sg": sg_all.astype(bf16),
        "crhs": crhs, "labels": labp, "ident": ident, "recb": recb,
    }
    in_maps = []
    for c in range(NCORES):
        rows = slice(BSH * c, BSH * (c + 1))
        a2t_c = np.ascontiguousarray(
            a2[rows].T.reshape(DCH, 128, BSH))                     # [DCH,128,BSH]
        clhs_c = np.empty((2, BSH), dtype=np.float32)
        clhs_c[0] = kb - wa[rows]
        clhs_c[1] = 1.0
        in_maps.append(dict(shared, a2t=a2t_c, clhs=clhs_c))
    return in_maps


def kernel(**inputs) -> np.ndarray:
    from concourse.bass_utils import run_bass_kernel_spmd

    if "nc" not in _CACHE:
        _CACHE["nc"] = _build_nc()
    nc = _CACHE["nc"]

    in_maps = _prep_host(
        inputs["inputs"], inputs["support_tensors"], inputs["support_labels"],
        inputs["kernel_w"], inputs["kernel_b"],
    )
    res = run_bass_kernel_spmd(nc, in_maps, list(range(NCORES)))
    return np.concatenate([res.results[i]["out"] for i in range(NCORES)], axis=0)


# revision 27
# speedup vs baseline: 1.1112x; 1.0869x over previous
"""Trainium2 Bass kernel for the siamese-kNN classification head.

Reference computation (B=256, N=2000, D=512, C=100):
    scores[b,n] = sigmoid(sum_d w_d * |a[b,d] - S[n,d]| + kb)
    out[b,c]    = (scores @ L)[b,c] / count_c     (0 where count_c == 0)

Strategy
--------
Data-parallel over the batch: core i handles rows 32*i .. 32*i+32, no
collectives.  Via |x| = relu(2x) - x the score splits into a nonlinear slab
relu(A''-S'') (A''=2|w|a, S''=2|w|S, bf16) plus a separable rank-2 linear
correction preloaded into PSUM (kb - w.a_b + (w.S)_n, f32r matmul).

Layout: d on partitions (4 chunks of 128), n on the free dim.  Slabs are
produced by DVE (tensor_scalar, 4x bf16 mode, ~0.74us/slab) and ACT
(activation Relu, ~1.9us/slab); the PE reduces each slab over d with a
[128,32] sliding-window stationary that is zero except column 31-b = -/+ the
per-row sign of w, accumulating row b of psc[32, seg] in PSUM.

Main optimizations over the v1 kernel (133.3us):
 1. The d-axis is HOST-PERMUTED so every partition row of chunk pair (0,1)
    and (2,3) carries two SAME-SIGN w's (sign classes padded to even size by
    reclassifying the smallest-|w| d, error ~1e-4).  For "paired" batch rows
    the two chunk slabs are then pair-added on DVE (tensor_tensor, 2x bf16)
    and the PE ingests HALF the data with a per-pair-row sign stationary.
    This rebalances the engines: PE was the 115us bottleneck with DVE at 83
    and ACT at 34; now PE/DVE/ACT all land near ~91us.
 2. More batch rows produced on ACT (11 vs 4), all of them paired.
 3. Startup: DMA issue is split across both HWDGE queues (Sync + ACT) in
    priority order, the rank-2 correction matmuls run while s2t streams in
    (they only need the tiny clhs/crhs), and phase A only touches chunk 0,
    so the PE starts ~3us earlier.
 4. PSUM segments are [512,512,512,256,208] (128-aligned): no transpose
    window straddles segments, and the small trailing segments shorten the
    close->sigmoid->transpose->label-matmul tail, which is pipelined
    per-segment.
"""

import sys

for _p in ("/opt/trn_rl_repo", "/root/.axon_site/_ro/trn_rl_repo"):
    if _p not in sys.path:
        sys.path.append(_p)

import numpy as np

B, N, D, C = 256, 2000, 512, 100
NP = 2048                  # label rows padded to 16 full chunks
NCORES = 8
BSH = B // NCORES          # 32 batch rows per core
DCH = D // 128             # 4 d-chunks
SEGL = [512, 512, 512, 256, 208]       # PSUM free-dim segments (128-aligned)
SEGO = [0, 512, 1024, 1536, 1792]      # segment start offsets
NSEG = len(SEGL)
NLAB = NP // 128           # 16 label chunks
WIN_SEG = [0, 0, 0, 0, 1, 1, 1, 1, 2, 2, 2, 2, 3, 3, 4, 4]  # 128-col window -> seg

N_ACT = 11                 # batch rows produced on ACT (always paired)
N_PAIR_DVE = 1             # DVE-produced rows that are also pair-added
# Offloading pair-adds failed twice: GpSimd TENSOR ops share the SBUF port
# with DVE (DVE slabs 735->1274ns, kernel 117->181us at 20 gp adds), and
# swdge accumulating DMAs have ~5-6us latency after the second slab (PE
# stalls at every dma-paired group: 132us at 8 adds, 135us at 34).  All
# pair-adds stay on DVE.
DMA_ADD_FRAC = (0, 1)      # fraction of pair-adds routed to swdge accum-DMA
PEND_KEEP = 2              # pair MM deferral depth
N_TAILB = 3                # trailing rows emitted seg-major to stagger closes

_CACHE = {}


def _sets():
    acts = [b for b in range(32)
            if (b * N_ACT) // BSH != ((b + 1) * N_ACT) // BSH]
    dves = [b for b in range(32) if b not in acts]
    # paired DVE rows from the middle of the DVE list
    m = len(dves) // 2
    pdve = dves[m:m + N_PAIR_DVE]
    unp = [b for b in dves if b not in pdve]
    return set(acts), acts + pdve, unp


def _split_multi_waits(nc):
    """TRN2 TPB instructions encode at most ONE semaphore wait, but Tile can
    attach several; split the extras into single-wait NOPs directly before
    the instruction on the same engine (engines execute in order)."""
    from concourse import mybir

    for fn in nc.m.functions:
        for bb in fn.blocks:
            out = []
            for inst in bb.instructions:
                si = inst.sync_info
                if si is not None and si.on_wait and len(si.on_wait) > 1:
                    waits = list(si.on_wait)
                    for j, w in enumerate(waits[:-1]):
                        out.append(mybir.InstNoOp(
                            name=f"{inst.name}-sw{j}", engine=inst.engine,
                            sync_info=mybir.SyncInfo(on_wait=[w], on_update=[]),
                            ins=[], outs=[]))
                    inst.sync_info = mybir.SyncInfo(
                        on_wait=[waits[-1]], on_update=list(si.on_update))
                out.append(inst)
            bb.instructions = out


def _interleave(xs, ys):
    """Merge two lists proportionally (xs items spread among ys, ys lead)."""
    out, i, j = [], 0, 0
    while i < len(xs) or j < len(ys):
        if j >= len(ys) or (i < len(xs) and
                            i * len(ys) < j * len(xs)):
            out.append(("x", xs[i])); i += 1
        else:
            out.append(("y", ys[j])); j += 1
    return out


def _build_nc():
    import concourse.bass as bass
    import concourse.tile as tile
    from concourse import mybir

    f32 = mybir.dt.float32
    f32r = mybir.dt.float32r
    bf16 = mybir.dt.bfloat16
    nc = bass.Bass()

    s2t_d = nc.declare_dram_parameter("s2t", [D, N], bf16, isOutput=False)
    a2t_d = nc.declare_dram_parameter("a2t", [DCH, 128, BSH], f32, isOutput=False)
    # sign tables, all packed in one tensor: [128, 12, 63]
    #   [:, 0:4]  = -sign per (row, chunk)     (DVE slabs)
    #   [:, 4:8]  = +sign per (row, chunk)     (ACT slabs)
    #   [:, 8:10] = -sign per (row, pairchunk) (DVE pair slabs)
    #   [:, 10:12]= +sign per (row, pairchunk) (ACT pair slabs)
    sg_d = nc.declare_dram_parameter("sg", [128, 12, 63], bf16, isOutput=False)
    clhs_d = nc.declare_dram_parameter("clhs", [2, BSH], f32r, isOutput=False)
    crhs_d = nc.declare_dram_parameter("crhs", [2, N], f32r, isOutput=False)
    lab_d = nc.declare_dram_parameter("labels", [128, NLAB, C], bf16, isOutput=False)
    ident_d = nc.declare_dram_parameter("ident", [32, 32], bf16, isOutput=False)
    recb_d = nc.declare_dram_parameter("recb", [BSH, C], f32, isOutput=False)
    out_d = nc.declare_dram_parameter("out", [BSH, C], f32, isOutput=True)

    acts, paired, unpaired = _sets()

    with tile.TileContext(nc) as tc:
        with (
            tc.tile_pool(name="const", bufs=1) as const,
            tc.tile_pool(name="dslab", bufs=10) as dpool,
            tc.tile_pool(name="aslab", bufs=6) as apool,
            tc.tile_pool(name="qslab", bufs=8) as qpool,
            tc.tile_pool(name="bank", bufs=8, space="PSUM") as bankp,
        ):
            # ---- constant loads: split across both HWDGE queues (Sync+ACT),
            # smallest/most-critical first.  Each dma_start costs ~650ns of
            # serial issue time on its queue.
            clhs = const.tile([2, BSH], f32r, name="clhs", tag="clhs")
            nc.sync.dma_start(clhs[:], clhs_d[:])
            crhs = const.tile([2, N], f32r, name="crhs", tag="crhs")
            nc.scalar.dma_start(crhs[:], crhs_d[:])
            # chunk-0 tile filled by four parallel DMAs (two per queue)
            s2tv = [
                const.tile([128, N], bf16, name=f"s2t{c}", tag=f"s2t{c}")
                for c in range(DCH)
            ]
            nc.sync.dma_start(s2tv[0][:, 0:512], s2t_d[0:128, 0:512])
            a2t = const.tile([128, DCH * BSH], f32, name="a2t", tag="a2t")
            nc.scalar.dma_start(
                a2t[:].rearrange("p (c b) -> p c b", c=DCH),
                a2t_d[:].rearrange("c p b -> p c b"),
            )
            nc.sync.dma_start(s2tv[0][:, 1024:1536], s2t_d[0:128, 1024:1536])
            nc.scalar.dma_start(s2tv[0][:, 512:1024], s2t_d[0:128, 512:1024])
            nc.sync.dma_start(s2tv[1][:], s2t_d[128:256, :])
            nc.scalar.dma_start(s2tv[0][:, 1536:N], s2t_d[0:128, 1536:N])
            sg = const.tile([128, 12, 63], bf16, name="sg", tag="sg")
            nc.scalar.dma_start(sg[:], sg_d[:])
            nc.sync.dma_start(s2tv[3][:], s2t_d[384:512, :])
            nc.scalar.dma_start(s2tv[2][:], s2t_d[256:384, :])
            ident = const.tile([32, 32], bf16, name="ident", tag="ident")
            nc.scalar.dma_start(ident[:], ident_d[:])
            labs = const.tile([128, NLAB, C], bf16, name="labs", tag="labs")
            nc.sync.dma_start(labs[:], lab_d[:])
            recb = const.tile([BSH, C], f32, name="recb", tag="recb")
            nc.sync.dma_start(recb[:], recb_d[:])

            # ---- rank-2 corrections: first PE work, overlaps s2t transfer
            psc = [
                bankp.tile([BSH, SEGL[s]], f32, name=f"psc{s}", tag="bank")
                for s in range(NSEG)
            ]
            for s in range(NSEG):
                nc.tensor.matmul(
                    psc[s][:], clhs[:], crhs[:, SEGO[s]:SEGO[s] + SEGL[s]],
                    start=True, stop=False,
                )
            # ramp fillers: keep the PE p-state climbing while s2t chunk 0 is
            # still in flight (a >100ns idle drops the clock back to 1.2GHz
            # for the next 3us of work)
            scratch = bankp.tile([2, 512], f32, name="scratch", tag="bank")
            for _ in range(5):
                nc.tensor.matmul(
                    scratch[:], clhs[:, 0:2], crhs[:, 0:512],
                    start=True, stop=True, skip_group_check=True,
                )

            def produce(ch, b, into=None):
                if b in acts:
                    slab = into if into is not None else apool.tile(
                        [128, N], bf16, name="aslab", tag="aslab")
                    nc.scalar.activation(
                        slab[:], s2tv[ch][:],
                        mybir.ActivationFunctionType.Relu,
                        bias=a2t[:, ch * BSH + b: ch * BSH + b + 1],
                        scale=-1.0,
                    )
                else:
                    slab = into if into is not None else dpool.tile(
                        [128, N], bf16, name="dslab", tag="dslab")
                    nc.vector.tensor_scalar(
                        slab[:], s2tv[ch][:],
                        a2t[:, ch * BSH + b: ch * BSH + b + 1], 0.0,
                        mybir.AluOpType.subtract, mybir.AluOpType.min,
                    )
                return slab

            def emit_mms(slab, tcol, b, last=False):
                """5 segment matmuls reducing `slab` into psc via the sliding
                sign window in column tcol of the sign-table tile."""
                lhs = sg[:, tcol, 31 - b: 63 - b]
                for s in range(NSEG):
                    nc.tensor.matmul(
                        psc[s][:], lhs,
                        slab[:, SEGO[s]:SEGO[s] + SEGL[s]],
                        start=False, stop=last,
                        skip_group_check=True,
                    )

            pend = []
            nadd = [0]

            def flush(keep=0):
                while len(pend) > keep:
                    q, tcol, b = pend.pop(0)
                    emit_mms(q, tcol, b)

            def emit_pair(cA, cB, b):
                """Produce both chunk slabs and combine them into one q tile,
                via swdge accumulating DMA (idle fabric) or DVE tensor_add."""
                i, (num, den) = nadd[0], DMA_ADD_FRAC
                nadd[0] += 1
                if (i * num) // den != ((i + 1) * num) // den:
                    sA = produce(cA, b)
                    q = qpool.tile([128, N], bf16, name="qslab", tag="qslab")
                    produce(cB, b, into=q)
                    nc.gpsimd.dma_start(q[:], sA[:],
                                        accum_op=mybir.AluOpType.add)
                else:
                    sA = produce(cA, b)
                    sB = produce(cB, b)
                    q = qpool.tile([128, N], bf16, name="qslab", tag="qslab")
                    nc.vector.tensor_add(q[:], sA[:], sB[:])
                return q

            # ---- main loop: phase A (unpaired ch0), then two chunk-pair
            # phases with paired rows interleaved among unpaired rows
            for b in unpaired:
                emit_mms(produce(0, b), 0, b)
            tailb = set(unpaired[-N_TAILB:])
            order = _interleave(paired, unpaired[:-1]) + [("y", unpaired[-1])]
            tail_groups = []
            for ph, (cA, cB) in enumerate([(0, 1), (2, 3)]):
                for kind, b in order:
                    if kind == "x":          # paired row: both chunks + add
                        q = emit_pair(cA, cB, b)
                        pend.append((q, (10 if b in acts else 8) + ph, b))
                    elif ph == 1:            # unpaired row: chunk 2 then 3
                        if b in tailb:       # defer: emitted seg-major below
                            tail_groups.append((produce(2, b), 2, b))
                            tail_groups.append((produce(3, b), 3, b))
                            if b == unpaired[-1]:
                                flush(0)
                            continue
                        slab = produce(2, b)
                        flush(PEND_KEEP)
                        emit_mms(slab, 2, b)
                        slab = produce(3, b)
                        flush(PEND_KEEP)
                        emit_mms(slab, 3, b)
                    else:                    # unpaired row: chunk 1
                        slab = produce(1, b)
                        flush(PEND_KEEP)
                        emit_mms(slab, 1, b)
            assert not pend, "pair groups left unflushed"
            # trailing groups seg-major: psc[s] closes stagger ~2us so the
            # serial sigmoid chain starts before the PE fully drains
            for s in range(NSEG):
                for gi, (slab, tcol, b) in enumerate(tail_groups):
                    nc.tensor.matmul(
                        psc[s][:], sg[:, tcol, 31 - b: 63 - b],
                        slab[:, SEGO[s]:SEGO[s] + SEGL[s]],
                        start=False, stop=(gi == len(tail_groups) - 1),
                        skip_group_check=True,
                    )

            # ---- tail: per-seg sigmoid -> transposes -> copies -> label MMs
            ssig = const.tile([BSH, N], bf16, name="ssig", tag="ssig")
            tpall = bankp.tile([128, NLAB * BSH], bf16, name="tpall", tag="bank")
            sct = const.tile([128, NLAB * BSH], bf16, name="sct", tag="sct")
            out_ps = bankp.tile([BSH, C], f32, name="out_ps", tag="bank")
            for s in range(NSEG):
                nc.scalar.activation(
                    ssig[:, SEGO[s]:SEGO[s] + SEGL[s]], psc[s][:],
                    mybir.ActivationFunctionType.Sigmoid,
                )
                wins = [k for k in range(NLAB) if WIN_SEG[k] == s]
                for k in wins:
                    pk = min(128, N - 128 * k)
                    nc.tensor.transpose(
                        tpall[:pk, BSH * k: BSH * k + BSH],
                        ssig[:, 128 * k: 128 * k + pk], ident[:],
                    )
                lo, hi = BSH * wins[0], BSH * wins[-1] + BSH
                nc.vector.tensor_copy(sct[:, lo:hi], tpall[:, lo:hi])
            # all 16 label matmuls after the transposes: windows 0-13 stream
            # while the seg-4 copy completes, instead of five serial
            # transpose->copy->matmul round-trips on the in-order PE
            for k in range(NLAB):
                pk = min(128, N - 128 * k)
                nc.tensor.matmul(
                    out_ps[:], sct[:pk, BSH * k: BSH * k + BSH],
                    labs[:pk, k, :],
                    start=(k == 0), stop=(k == NLAB - 1),
                )

            # ---- divide by counts, write out ----
            out_s = const.tile([BSH, C], f32, name="out_s", tag="out_s")
            nc.vector.tensor_mul(out_s[:], out_ps[:], recb[:])
            nc.sync.dma_start(out_d[:], out_s[:])

    _split_multi_waits(nc)
    return nc


def _prep_host(inputs, support_tensors, support_labels, kernel_w, kernel_b):
    import ml_dtypes

    bf16 = ml_dtypes.bfloat16
    a = np.asarray(inputs, dtype=np.float32)
    S = np.asarray(support_tensors, dtype=np.float32)
    L = np.asarray(support_labels, dtype=np.float32)
    w = np.asarray(kernel_w, dtype=np.float32)
    kb = np.float32(np.asarray(kernel_b, dtype=np.float32))

    # ---- sign classes, padded to even sizes by reclassifying the smallest
    # |w| element of the odd class (relu-sign error ~|w_min|, negligible)
    sgn = np.where(w >= 0, 1.0, -1.0).astype(np.float32)
    pos = np.where(sgn > 0)[0]
    if len(pos) % 2 == 1:
        mv = pos[np.argmin(np.abs(w[pos]))]
        sgn[mv] = -1.0
    pos = np.where(sgn > 0)[0]
    neg = np.where(sgn < 0)[0]
    # pairs of same-sign d's; 256 total
    pairs = np.concatenate([pos.reshape(-1, 2), neg.reshape(-1, 2)], axis=0)
    assert pairs.shape == (256, 2)
    # permutation: chunk0 = pairs[0:128,0], chunk1 = pairs[0:128,1], etc.
    perm = np.concatenate([
        pairs[0:128, 0], pairs[0:128, 1], pairs[128:256, 0], pairs[128:256, 1]
    ])
    assert np.array_equal(np.sort(perm), np.arange(D))

    aw = 2.0 * np.abs(w)
    s2t = np.ascontiguousarray((S * aw[None, :]).T[perm]).astype(bf16)  # [D, N]
    wS = (S @ w).astype(np.float32)                                # [N]
    wa = (a @ w).astype(np.float32)                                # [B]
    a2 = (a * aw[None, :])[:, perm]                                # [B, D] permuted

    # sign tables [128, 12, 63]: col 31 = value, layout per _build_nc
    sgn_chunks = sgn[perm].reshape(DCH, 128).T                     # [128, 4]
    pair_sgn = sgn[pairs[:, 0]].reshape(2, 128).T                  # [128, 2]
    sg_all = np.zeros((128, 12, 63), dtype=np.float32)
    sg_all[:, 0:4, 31] = -sgn_chunks
    sg_all[:, 4:8, 31] = sgn_chunks
    sg_all[:, 8:10, 31] = -pair_sgn
    sg_all[:, 10:12, 31] = pair_sgn

    crhs = np.empty((2, N), dtype=np.float32)
    crhs[0] = 1.0
    crhs[1] = wS
    labp = np.zeros((NP, C), dtype=np.float32)
    labp[:N] = L
    labp = np.ascontiguousarray(
        labp.reshape(NLAB, 128, C).transpose(1, 0, 2)).astype(bf16)
    ident = np.eye(32, dtype=bf16)
    counts = L.sum(axis=0)
    recip = np.where(counts != 0, 1.0 / np.maximum(counts, 1e-30), 0.0)
    recb = np.broadcast_to(recip.astype(np.float32), (BSH, C)).copy()

    shared = {
        "s2t": s2t, "sg": sg_all.astype(bf16),
        "crhs": crhs, "labels": labp, "ident": ident, "recb": recb,
    }
    in_maps = []
    for c in range(NCORES):
        rows = slice(BSH * c, BSH * (c + 1))
        a2t_c = np.ascontiguousarray(
            a2[rows].T.reshape(DCH, 128, BSH))                     # [DCH,128,BSH]
        clhs_c = np.empty((2, BSH), dtype=np.float32)
        clhs_c[0] = kb - wa[rows]
        clhs_c[1] = 1.0
        in_maps.append(dict(shared, a2t=a2t_c, clhs=clhs_c))
    return in_maps


def kernel(**inputs) -> np.ndarray:
    from concourse.bass_utils import run_bass_kernel_spmd

    if "nc" not in _CACHE:
        _CACHE["nc"] = _build_nc()
    nc = _CACHE["nc"]

    in_maps = _prep_host(
        inputs["inputs"], inputs["support_tensors"], inputs["support_labels"],
        inputs["kernel_w"], inputs["kernel_b"],
    )
    res = run_bass_kernel_spmd(nc, in_maps, list(range(NCORES)))
    return np.concatenate([res.results[i]["out"] for i in range(NCORES)], axis=0)
